# revision 8
# baseline (speedup 1.0000x reference)
"""Multi-head attention layer for Trainium2, 8 NeuronCores.

Problem (hardcoded): B=4, S=2048, D=1024, H=16 heads, DH=64.
  q,k,v = x@W* + b*;  scores = (q k^T)/sqrt(DH) - 10000*(1-mask_k);
  out = softmax(scores) @ v, heads concatenated.

Sharding: 8 cores = (batch b in 0..3) x (head-group g in 0..1).
Each core handles one batch element and 8 heads (512 of the 1024 output
channels), so outputs are disjoint and no collectives are needed.

Host-side prep (numpy layout work, not device FLOPs): x is transposed,
cast to fp16 and laid out chunk-major so every DMA is a contiguous
per-partition stream; W* likewise. This removes all on-device PE
transposes/casts and makes input DMA run at full HBM bandwidth.

Per-core kernel (the scalar engine's Exp at 1 elem/cycle/lane is the
bottleneck; all PE/DVE work is scheduled to hide under it):
  1. V' [s, dout|1] = xT.T @ Wv (+bias via rank-1 ones matmul; the ones
     column accumulates the softmax denominator during PV).
     QT/KT [dout, s] = W.T @ xT (bias added on PSUM->SBUF copy-out).
     Only the first half of V and K/Q for the first queries are
     projected up front; the rest streams inside early attention.
  2. Per head-pair, per 512-query chunk, per 128-key tile kt (fully
     masked key tiles are skipped - their exp is exactly 0):
     scoresT[k,q] for both heads in one PE pass (row-group tiling);
     expT = Exp(0.125*scoresT + maskbias_k) on the scalar engine;
     h'T[dd,q] += V'[k,dd].T @ expT  (row 64 = sum of exp = denom).
  3. h'T transposed back on the PE (spread one query-tile per kt step
     of the next chunk); h = h'T[0:64]/h'T[64], both heads coalesced
     into one [128,128] tile per query tile, DMA'd out.
"""
import numpy as np
from contextlib import ExitStack
from collections import deque
from itertools import chain

import concourse.bass as bass
import concourse.bacc as bacc
import concourse.mybir as mybir
from concourse.tile import TileContext
from concourse.bass_utils import run_bass_kernel_spmd
from concourse.masks import make_identity

B, S, D, H = 4, 2048, 1024, 16
DH = 64
HPC = 8            # heads per core
DC = HPC * DH      # 512 output channels per core
KT_D = D // 128    # 8 contraction tiles over d_in
MT = DC // 128     # 4 tiles over local d_out
ST = S // 128      # 16 s-tiles
QCH = S // 512     # 4 query chunks
NCORES = 8

FP32 = mybir.dt.float32
FP16 = mybir.dt.float16
AFT = mybir.ActivationFunctionType


def build_kernel(active_kt):
    """active_kt: sorted tuple of key-tile indices with any unmasked key.
    Fully-masked tiles contribute exp(s-10000)==0 exactly, so they are
    skipped in scores/exp/PV and V projection."""
    active_kt = list(active_kt)
    nc = bacc.Bacc("TRN2", target_bir_lowering=False, debug=False)
    # host-packed layouts (see make_in_maps)
    xt_d = nc.dram_tensor("xt", (QCH, 128, KT_D * 512), FP16, kind="ExternalInput")
    mask_d = nc.dram_tensor("mask", (128, ST), FP32, kind="ExternalInput")
    wq_d = nc.dram_tensor("wq", (128, KT_D * DC), FP16, kind="ExternalInput")
    wk_d = nc.dram_tensor("wk", (128, KT_D * DC), FP16, kind="ExternalInput")
    wv_d = nc.dram_tensor("wv", (128, KT_D * DC), FP16, kind="ExternalInput")
    bq_d = nc.dram_tensor("bq", (128, MT), FP32, kind="ExternalInput")
    bk_d = nc.dram_tensor("bk", (128, MT), FP32, kind="ExternalInput")
    bv_d = nc.dram_tensor("bv", (DC,), FP32, kind="ExternalInput")
    out_d = nc.dram_tensor("out", (S, DC), FP32, kind="ExternalOutput")

    with TileContext(nc) as tc, ExitStack() as ctx:
        const = ctx.enter_context(tc.tile_pool(name="const", bufs=1))
        big = ctx.enter_context(tc.tile_pool(name="big", bufs=1))
        exp_pool = ctx.enter_context(tc.tile_pool(name="expp", bufs=5))
        ht_pool = ctx.enter_context(tc.tile_pool(name="htp", bufs=2))
        o_pool = ctx.enter_context(tc.tile_pool(name="op", bufs=2))
        ps_pool = ctx.enter_context(
            tc.tile_pool(name="psp", bufs=2, space=bass.MemorySpace.PSUM))
        psh_pool = ctx.enter_context(
            tc.tile_pool(name="pshp", bufs=2, space=bass.MemorySpace.PSUM))
        pst_pool = ctx.enter_context(
            tc.tile_pool(name="pstp", bufs=2, space=bass.MemorySpace.PSUM))

        # ---- input DMAs (queue order: wv + x chunk 0 first so V
        # projection starts immediately) ----
        mask_sb = const.tile([128, ST], FP32)
        nc.sync.dma_start(mask_sb[:], mask_d[:])
        bq_sb = const.tile([128, MT], FP32)
        bk_sb = const.tile([128, MT], FP32)
        bv_f = const.tile([1, DC], FP32)
        nc.sync.dma_start(bq_sb[:], bq_d[:])
        nc.sync.dma_start(bk_sb[:], bk_d[:])
        nc.sync.dma_start(bv_f[:], bv_d[None, :])

        wv_sb = big.tile([128, KT_D, DC], FP16)
        wk_sb = big.tile([128, KT_D, DC], FP16)
        wq_sb = big.tile([128, KT_D, DC], FP16)
        xt_sb = big.tile([128, KT_D, S], FP16)
        nc.sync.dma_start(wv_sb[:], wv_d[:].rearrange("p (k n) -> p k n", n=DC))
        nc.sync.dma_start(
            xt_sb[:, :, 0:512],
            xt_d[0].rearrange("p (k s) -> p k s", s=512))
        nc.sync.dma_start(wk_sb[:], wk_d[:].rearrange("p (k n) -> p k n", n=DC))
        for qch in range(1, QCH):
            nc.sync.dma_start(
                xt_sb[:, :, qch * 512:(qch + 1) * 512],
                xt_d[qch].rearrange("p (k s) -> p k s", s=512))
        nc.sync.dma_start(wq_sb[:], wq_d[:].rearrange("p (k n) -> p k n", n=DC))

        # ---- consts ----
        ident = const.tile([128, 128], FP32)
        make_identity(nc, ident[:])
        ones_r = const.tile([1, 128], FP16)
        nc.vector.memset(ones_r[:], 1.0)
        bv_row = const.tile([1, DC], FP16)
        nc.vector.tensor_copy(bv_row[:], bv_f[:])
        # additive bias per key position: -10000*(1-mask)
        kbias = const.tile([128, ST], FP32)
        nc.vector.tensor_scalar(kbias[:], mask_sb[:], -1.0, 10000.0,
                                mybir.AluOpType.add, mybir.AluOpType.mult)

        # persistent activations
        qt_sb = big.tile([128, MT, S], FP16)           # QT: [dout, s]
        kt_sb = big.tile([128, MT, S], FP16)           # KT: [dout, s]
        v_sb = big.tile([128, ST, HPC, DH + 1], FP16)  # V': [s_p, s_t, head, d|1]
        nc.vector.memset(v_sb[:, :, :, DH:DH + 1], 1.0)

        def v_tile(st, pool_tag):
            pool = ps_pool if pool_tag == "ps" else pst_pool
            ps = pool.tile([128, 512], FP32, tag=pool_tag,
                           bufs=1 if pool_tag == "pacc" else None)
            for ktc in range(KT_D):
                nc.tensor.matmul(
                    ps[:],
                    xt_sb[:, ktc, st * 128:(st + 1) * 128],
                    wv_sb[:, ktc, :],
                    start=(ktc == 0), stop=False)
                if pool_tag == "pacc" and ktc in (2, 5):
                    yield
            nc.tensor.matmul(ps[:], ones_r[:], bv_row[:], start=False, stop=True)
            nc.vector.tensor_copy(
                v_sb[:, st, :, 0:DH],
                ps[:].rearrange("p (h d) -> p h d", d=DH))
            if pool_tag == "pacc":
                yield

        K, Q = 0, 1

        def proj_tile(mt, which, qch, pool_tag):
            pool = ps_pool if pool_tag == "ps" else pst_pool
            w_sb, b_sb, dst = ((wk_sb, bk_sb, kt_sb), (wq_sb, bq_sb, qt_sb))[which]
            ps = pool.tile([128, 512], FP32, tag=pool_tag,
                           bufs=1 if pool_tag == "pacc" else None)
            for ktc in range(KT_D):
                nc.tensor.matmul(
                    ps[:],
                    w_sb[:, ktc, mt * 128:(mt + 1) * 128],
                    xt_sb[:, ktc, qch * 512:(qch + 1) * 512],
                    start=(ktc == 0), stop=(ktc == KT_D - 1))
                if pool_tag == "pacc" and ktc in (2, 5):
                    yield
            nc.vector.tensor_scalar_add(
                dst[:, mt, qch * 512:(qch + 1) * 512],
                ps[:], b_sb[:, mt:mt + 1])
            if pool_tag == "pacc":
                yield

        # ---- up-front projections: V first half, K(0, 0..1), Q(0, 0) ----
        upfront_v = [st for st in active_kt if st < 8]
        stream_v = [st for st in active_kt if st >= 8]
        for st in upfront_v:
            for _ in v_tile(st, "ps"):
                pass
        for qch in (0, 1):
            for _ in proj_tile(0, K, qch, "ps"):
                pass
        for _ in proj_tile(0, Q, 0, "ps"):
            pass

        # streamed work (generators; "pacc" psum accumulator, bufs=1):
        #  qc0 of pair 0: rest of V + K(0,2..3) + Q(0,1..3), availability-
        #  ordered. pair p: K/Q of pair p+1.
        def ramp_gen():
            items = []
            sv = list(stream_v)
            # interleave: 2 V tiles, then a K chunk, ...
            items += [("v", st) for st in sv[0:2]]
            items.append(("p", (0, K, 2)))
            items += [("v", st) for st in sv[2:5]]
            items.append(("p", (0, K, 3)))
            items += [("v", st) for st in sv[5:]]
            items += [("p", (0, Q, c)) for c in range(1, QCH)]
            for kind, arg in items:
                if kind == "v":
                    yield from v_tile(arg, "pacc")
                else:
                    yield from proj_tile(*arg, "pacc")

        def pair_gen(p):
            for which in (K, Q):
                for c in range(QCH):
                    yield from proj_tile(p, which, c, "pacc")

        # ---- deferred epilogue: at qc end, copy h' to SBUF; the
        # transpose/divide/DMA pieces run one query-tile per kt step of
        # the NEXT qc so the PE burst never starves the scalar engine.
        pend_epi = []

        def epi_piece():
            # one head x query-tile per call (8 calls per finished qc)
            if not pend_epi:
                return
            st = pend_epi[0]
            epair, eq0, htA, htB, k, o_sb = st
            qt, hsel = k >> 1, k & 1
            ht = htB if hsel else htA
            tp = pst_pool.tile([128, DH + 1], FP32, tag="tp", bufs=1)
            nc.tensor.transpose(tp[:], ht[:, qt * 128:(qt + 1) * 128],
                                ident[0:DH + 1, 0:DH + 1])
            rec = o_pool.tile([128, 1], FP32, tag="rec")
            nc.vector.reciprocal(rec[:], tp[:, DH:DH + 1])
            if not hsel:
                o_sb = o_pool.tile([128, 128], FP32, tag="o")
            nc.vector.tensor_scalar_mul(
                o_sb[:, hsel * DH:(hsel + 1) * DH], tp[:, 0:DH], rec[:])
            if hsel:
                row = eq0 + qt * 128
                nc.sync.dma_start(
                    out_d[row:row + 128, epair * 128:(epair + 1) * 128], o_sb[:])
            if k == 7:
                pend_epi.pop(0)
            else:
                pend_epi[0] = (epair, eq0, htA, htB, k + 1, o_sb)

        def qc_done(pair, q0, hA, hB):
            htA = ht_pool.tile([DH + 1, 512], FP32, tag="ht")
            nc.vector.tensor_copy(htA[:], hA[:])
            htB = ht_pool.tile([DH + 1, 512], FP32, tag="ht")
            nc.vector.tensor_copy(htB[:], hB[:])
            pend_epi.append([pair, q0, htA, htB, 0, None])

        first_kt = active_kt[0]
        for pair in range(HPC // 2):
            gen_rest = pair_gen(pair + 1) if pair < HPC // 2 - 1 else iter(())
            for qc in range(QCH):
                is_ramp = (pair == 0 and qc == 0)
                g = ramp_gen() if is_ramp else gen_rest
                nsub = 3 if is_ramp else 1
                q0 = qc * 512
                hA = psh_pool.tile([DH + 1, 512], FP32, tag="h")
                hB = psh_pool.tile([DH + 1, 512], FP32, tag="h")
                # software pipeline (depth 2): pv(kt-2) is emitted after
                # scores(kt) so the exp never waits behind PV/projection.
                pend = deque()

                def flush_pv(last=False):
                    pkt, pe = pend.popleft()
                    nc.tensor.matmul(hA[:], v_sb[:, pkt, 2 * pair, :],
                                     pe[:, 0:512],
                                     start=(pkt == first_kt),
                                     stop=last and not pend)
                    nc.tensor.matmul(hB[:], v_sb[:, pkt, 2 * pair + 1, :],
                                     pe[:, 512:1024],
                                     start=(pkt == first_kt),
                                     stop=last and not pend)

                for idx, kt in enumerate(active_kt):
                    k0 = kt * 128
                    scAB = ps_pool.tile([128, 1024], FP32, tag="ps")
                    nc.tensor.matmul(scAB[:, 0:512],
                                     kt_sb[0:64, pair, k0:k0 + 128],
                                     qt_sb[0:64, pair, q0:q0 + 512],
                                     start=True, stop=True)
                    nc.tensor.matmul(scAB[:, 512:1024],
                                     kt_sb[64:128, pair, k0:k0 + 128],
                                     qt_sb[64:128, pair, q0:q0 + 512],
                                     start=True, stop=True)
                    eAB = exp_pool.tile([128, 1024], FP16, tag="exp")
                    nc.scalar.activation(eAB[:], scAB[:], AFT.Exp,
                                         bias=kbias[:, kt:kt + 1], scale=0.125)
                    if len(pend) >= 2:
                        flush_pv()
                    pend.append((kt, eAB))
                    if 1 <= idx <= 8:
                        epi_piece()
                    if idx >= 1:
                        for _ in range(nsub):
                            next(g, None)
                while pend:
                    flush_pv(last=True)
                if is_ramp:
                    for _ in g:
                        pass
                qc_done(pair, q0, hA, hB)
            for _ in gen_rest:
                pass
        while pend_epi:
            epi_piece()

    nc.compile()
    return nc


_NC_CACHE = {}


def _get_nc(active_kt):
    key = tuple(active_kt)
    if key not in _NC_CACHE:
        _NC_CACHE[key] = build_kernel(key)
    return _NC_CACHE[key]


def _active_kt(mask):
    # key tile kt is skippable iff masked out in EVERY batch row (the
    # same compiled program runs on all cores).
    m = np.asarray(mask, dtype=np.float32).reshape(B, ST, 128)
    return tuple(t for t in range(ST) if np.any(m[:, t, :] != 0.0))


def _pack_w(W, cs):
    # [D, DC] -> [128, KT_D*DC]: partition p holds rows {kt*128+p}
    w = np.asarray(W)[:, cs].astype(np.float16)
    return np.ascontiguousarray(
        w.reshape(KT_D, 128, DC).transpose(1, 0, 2).reshape(128, KT_D * DC))


def make_in_maps(x, mask, Wq, bq, Wk, bk, Wv, bv):
    asc = np.ascontiguousarray
    x = np.asarray(x)
    mask = np.asarray(mask)
    in_maps = []
    for c in range(NCORES):
        b, g = divmod(c, 2)
        cs = slice(g * DC, (g + 1) * DC)
        xt = x[b].T.astype(np.float16)  # [D, S]
        xt = xt.reshape(KT_D, 128, QCH, 512).transpose(2, 1, 0, 3)
        in_maps.append({
            "xt": asc(xt.reshape(QCH, 128, KT_D * 512)),
            "mask": asc(mask[b].reshape(ST, 128).T, dtype=np.float32),
            "wq": _pack_w(Wq, cs),
            "wk": _pack_w(Wk, cs),
            "wv": _pack_w(Wv, cs),
            "bq": asc(np.asarray(bq)[cs].reshape(MT, 128).T, dtype=np.float32),
            "bk": asc(np.asarray(bk)[cs].reshape(MT, 128).T, dtype=np.float32),
            "bv": asc(bv[cs], dtype=np.float32),
        })
    return in_maps


def kernel(x, mask, Wq, bq, Wk, bk, Wv, bv):
    nc = _get_nc(_active_kt(mask))
    in_maps = make_in_maps(x, mask, Wq, bq, Wk, bk, Wv, bv)
    res = run_bass_kernel_spmd(nc, in_maps, core_ids=list(range(NCORES)))
    out = np.empty((B, S, D), dtype=np.float32)
    for c in range(NCORES):
        b, g = divmod(c, 2)
        out[b, :, g * DC:(g + 1) * DC] = res.results[c]["out"]
    return out


# revision 9
# speedup vs baseline: 1.1547x; 1.1547x over previous
"""Multi-head attention layer for Trainium2, 8 NeuronCores.

Problem (hardcoded): B=4, S=2048, D=1024, H=16 heads, DH=64.
  q,k,v = x@W* + b*;  scores = (q k^T)/sqrt(DH) - 10000*(1-mask_k);
  out = softmax(scores) @ v, heads concatenated.

Sharding: 8 cores = (batch b in 0..3) x (head-group g in 0..1).
Each core handles one batch element and 8 heads (512 of the 1024 output
channels), so outputs are disjoint and no collectives are needed.

Host-side prep (numpy layout work, not device FLOPs): x is transposed,
cast to fp16 and laid out chunk-major so every DMA is a contiguous
per-partition stream; W* likewise. This removes all on-device PE
transposes/casts and makes input DMA run at full HBM bandwidth.

Per-core kernel (the scalar engine's Exp at 1 elem/cycle/lane is the
bottleneck; all PE/DVE work is scheduled to hide under it):
  1. V' [s, dout|1] = xT.T @ Wv (+bias via rank-1 ones matmul; the ones
     column accumulates the softmax denominator during PV).
     QT/KT [dout, s] = W.T @ xT (bias added on PSUM->SBUF copy-out).
     Only the first half of V and K/Q for the first queries are
     projected up front; the rest streams inside early attention.
  2. Per head-pair, per 512-query chunk, per 128-key tile kt (fully
     masked key tiles are skipped - their exp is exactly 0):
     scoresT[k,q] for both heads in one PE pass (row-group tiling);
     expT = Exp(0.125*scoresT + maskbias_k) on the scalar engine;
     h'T[dd,q] += V'[k,dd].T @ expT  (row 64 = sum of exp = denom).
  3. h'T transposed back on the PE (spread one query-tile per kt step
     of the next chunk); h = h'T[0:64]/h'T[64], both heads coalesced
     into one [128,128] tile per query tile, DMA'd out.
"""
import numpy as np
from contextlib import ExitStack
from collections import deque
from itertools import chain

import concourse.bass as bass
import concourse.bacc as bacc
import concourse.mybir as mybir
from concourse.tile import TileContext
from concourse.bass_utils import run_bass_kernel_spmd
from concourse.masks import make_identity

B, S, D, H = 4, 2048, 1024, 16
DH = 64
HPC = 8            # heads per core
DC = HPC * DH      # 512 output channels per core
KT_D = D // 128    # 8 contraction tiles over d_in
MT = DC // 128     # 4 tiles over local d_out
ST = S // 128      # 16 s-tiles
QCH = S // 512     # 4 query chunks
NCORES = 8

FP32 = mybir.dt.float32
FP16 = mybir.dt.float16
AFT = mybir.ActivationFunctionType


def build_kernel(active_kt):
    """active_kt: sorted tuple of key-tile indices with any unmasked key.
    Fully-masked tiles contribute exp(s-10000)==0 exactly, so they are
    skipped in scores/exp/PV and V projection."""
    active_kt = list(active_kt)
    nc = bacc.Bacc("TRN2", target_bir_lowering=False, debug=False)
    # host-packed layouts (see make_in_maps)
    xt_d = nc.dram_tensor("xt", (QCH, 128, KT_D * 512), FP16, kind="ExternalInput")
    mask_d = nc.dram_tensor("mask", (128, ST), FP32, kind="ExternalInput")
    wq_d = nc.dram_tensor("wq", (128, KT_D * DC), FP16, kind="ExternalInput")
    wk_d = nc.dram_tensor("wk", (128, KT_D * DC), FP16, kind="ExternalInput")
    wv_d = nc.dram_tensor("wv", (128, KT_D * DC), FP16, kind="ExternalInput")
    bq_d = nc.dram_tensor("bq", (128, MT), FP32, kind="ExternalInput")
    bk_d = nc.dram_tensor("bk", (128, MT), FP32, kind="ExternalInput")
    bv_d = nc.dram_tensor("bv", (DC,), FP32, kind="ExternalInput")
    out_d = nc.dram_tensor("out", (S, DC), FP32, kind="ExternalOutput")

    with TileContext(nc) as tc, ExitStack() as ctx:
        const = ctx.enter_context(tc.tile_pool(name="const", bufs=1))
        big = ctx.enter_context(tc.tile_pool(name="big", bufs=1))
        exp_pool = ctx.enter_context(tc.tile_pool(name="expp", bufs=5))
        ht_pool = ctx.enter_context(tc.tile_pool(name="htp", bufs=2))
        o_pool = ctx.enter_context(tc.tile_pool(name="op", bufs=2))
        ps_pool = ctx.enter_context(
            tc.tile_pool(name="psp", bufs=2, space=bass.MemorySpace.PSUM))
        psh_pool = ctx.enter_context(
            tc.tile_pool(name="pshp", bufs=2, space=bass.MemorySpace.PSUM))
        pst_pool = ctx.enter_context(
            tc.tile_pool(name="pstp", bufs=2, space=bass.MemorySpace.PSUM))

        # ---- input DMAs (queue order: wv + x chunk 0 first so V
        # projection starts immediately) ----
        mask_sb = const.tile([128, ST], FP32)
        nc.sync.dma_start(mask_sb[:], mask_d[:])
        bq_sb = const.tile([128, MT], FP32)
        bk_sb = const.tile([128, MT], FP32)
        bv_f = const.tile([1, DC], FP32)
        nc.sync.dma_start(bq_sb[:], bq_d[:])
        nc.sync.dma_start(bk_sb[:], bk_d[:])
        nc.sync.dma_start(bv_f[:], bv_d[None, :])

        wv_sb = big.tile([128, KT_D, DC], FP16)
        wk_sb = big.tile([128, KT_D, DC], FP16)
        wq_sb = big.tile([128, KT_D, DC], FP16)
        xt_sb = big.tile([128, KT_D, S], FP16)
        nc.sync.dma_start(wv_sb[:], wv_d[:].rearrange("p (k n) -> p k n", n=DC))
        nc.sync.dma_start(
            xt_sb[:, :, 0:512],
            xt_d[0].rearrange("p (k s) -> p k s", s=512))
        nc.sync.dma_start(wk_sb[:], wk_d[:].rearrange("p (k n) -> p k n", n=DC))
        for qch in range(1, QCH):
            nc.sync.dma_start(
                xt_sb[:, :, qch * 512:(qch + 1) * 512],
                xt_d[qch].rearrange("p (k s) -> p k s", s=512))
        nc.sync.dma_start(wq_sb[:], wq_d[:].rearrange("p (k n) -> p k n", n=DC))

        # ---- consts ----
        ident = const.tile([128, 128], FP32)
        make_identity(nc, ident[:])
        ones_r = const.tile([1, 128], FP16)
        nc.vector.memset(ones_r[:], 1.0)
        bv_row = const.tile([1, DC], FP16)
        nc.vector.tensor_copy(bv_row[:], bv_f[:])
        # additive bias per key position: -10000*(1-mask)
        kbias = const.tile([128, ST], FP32)
        nc.vector.tensor_scalar(kbias[:], mask_sb[:], -1.0, 10000.0,
                                mybir.AluOpType.add, mybir.AluOpType.mult)

        # persistent activations
        qt_sb = big.tile([128, MT, S], FP16)           # QT: [dout, s]
        kt_sb = big.tile([128, MT, S], FP16)           # KT: [dout, s]
        v_sb = big.tile([128, ST, HPC, DH + 1], FP16)  # V': [s_p, s_t, head, d|1]
        nc.vector.memset(v_sb[:, :, :, DH:DH + 1], 1.0)

        def v_tile(st, pool_tag):
            pool = ps_pool if pool_tag == "ps" else pst_pool
            ps = pool.tile([128, 512], FP32, tag=pool_tag)
            for ktc in range(KT_D):
                nc.tensor.matmul(
                    ps[:],
                    xt_sb[:, ktc, st * 128:(st + 1) * 128],
                    wv_sb[:, ktc, :],
                    start=(ktc == 0), stop=False)
                if pool_tag == "tp" and ktc in (2, 5):
                    yield
            nc.tensor.matmul(ps[:], ones_r[:], bv_row[:], start=False, stop=True)
            nc.vector.tensor_copy(
                v_sb[:, st, :, 0:DH],
                ps[:].rearrange("p (h d) -> p h d", d=DH))
            if pool_tag == "tp":
                yield

        K, Q = 0, 1

        def proj_tile(mt, which, qch, pool_tag):
            pool = ps_pool if pool_tag == "ps" else pst_pool
            w_sb, b_sb, dst = ((wk_sb, bk_sb, kt_sb), (wq_sb, bq_sb, qt_sb))[which]
            ps = pool.tile([128, 512], FP32, tag=pool_tag)
            for ktc in range(KT_D):
                nc.tensor.matmul(
                    ps[:],
                    w_sb[:, ktc, mt * 128:(mt + 1) * 128],
                    xt_sb[:, ktc, qch * 512:(qch + 1) * 512],
                    start=(ktc == 0), stop=(ktc == KT_D - 1))
                if pool_tag == "tp" and ktc in (2, 5):
                    yield
            nc.vector.tensor_scalar_add(
                dst[:, mt, qch * 512:(qch + 1) * 512],
                ps[:], b_sb[:, mt:mt + 1])
            if pool_tag == "tp":
                yield

        # ---- up-front projections: V first half, K(0, 0..1), Q(0, 0) ----
        upfront_v = [st for st in active_kt if st < 8]
        stream_v = [st for st in active_kt if st >= 8]
        for st in upfront_v:
            for _ in v_tile(st, "ps"):
                pass
        for qch in (0, 1):
            for _ in proj_tile(0, K, qch, "ps"):
                pass
        for _ in proj_tile(0, Q, 0, "ps"):
            pass

        # streamed work (generators; "pacc" psum accumulator, bufs=1):
        #  qc0 of pair 0: rest of V + K(0,2..3) + Q(0,1..3), availability-
        #  ordered. pair p: K/Q of pair p+1.
        def ramp_gen():
            items = []
            sv = list(stream_v)
            # interleave: 2 V tiles, then a K chunk, ...
            items += [("v", st) for st in sv[0:2]]
            items.append(("p", (0, K, 2)))
            items += [("v", st) for st in sv[2:5]]
            items.append(("p", (0, K, 3)))
            items += [("v", st) for st in sv[5:]]
            items += [("p", (0, Q, c)) for c in range(1, QCH)]
            for kind, arg in items:
                if kind == "v":
                    yield from v_tile(arg, "tp")
                else:
                    yield from proj_tile(*arg, "tp")

        def pair_gen(p):
            for which in (K, Q):
                for c in range(QCH):
                    yield from proj_tile(p, which, c, "tp")

        # ---- deferred epilogue: at qc end, copy h' to SBUF; the
        # transpose/divide/DMA pieces run one query-tile per kt step of
        # the NEXT qc so the PE burst never starves the scalar engine.
        pend_epi = []

        def epi_burst():
            if not pend_epi:
                return
            epair, eq0, htA, htB, _k, _o = pend_epi.pop(0)
            for qt in range(4):
                o_sb = o_pool.tile([128, 128], FP32, tag="o")
                for hsel, ht in ((0, htA), (1, htB)):
                    tp = pst_pool.tile([128, DH + 1], FP32, tag="tp")
                    nc.tensor.transpose(tp[:], ht[:, qt * 128:(qt + 1) * 128],
                                        ident[0:DH + 1, 0:DH + 1])
                    rec = o_pool.tile([128, 1], FP32, tag="rec")
                    nc.vector.reciprocal(rec[:], tp[:, DH:DH + 1])
                    nc.vector.tensor_scalar_mul(
                        o_sb[:, hsel * DH:(hsel + 1) * DH], tp[:, 0:DH], rec[:])
                row = eq0 + qt * 128
                nc.sync.dma_start(
                    out_d[row:row + 128, epair * 128:(epair + 1) * 128], o_sb[:])

        def qc_done(pair, q0, hA, hB):
            htA = ht_pool.tile([DH + 1, 512], FP32, tag="ht")
            nc.vector.tensor_copy(htA[:], hA[:])
            htB = ht_pool.tile([DH + 1, 512], FP32, tag="ht")
            nc.vector.tensor_copy(htB[:], hB[:])
            pend_epi.append([pair, q0, htA, htB, 0, None])

        first_kt = active_kt[0]
        for pair in range(HPC // 2):
            gen_rest = pair_gen(pair + 1) if pair < HPC // 2 - 1 else iter(())
            for qc in range(QCH):
                is_ramp = (pair == 0 and qc == 0)
                g = ramp_gen() if is_ramp else gen_rest
                nsub = 3 if is_ramp else 1
                q0 = qc * 512
                hA = psh_pool.tile([DH + 1, 512], FP32, tag="h")
                hB = psh_pool.tile([DH + 1, 512], FP32, tag="h")
                # software pipeline (depth 2): pv(kt-2) is emitted after
                # scores(kt) so the exp never waits behind PV/projection.
                pend = deque()

                def flush_pv(last=False):
                    pkt, pe = pend.popleft()
                    nc.tensor.matmul(hA[:], v_sb[:, pkt, 2 * pair, :],
                                     pe[:, 0:512],
                                     start=(pkt == first_kt),
                                     stop=last and not pend)
                    nc.tensor.matmul(hB[:], v_sb[:, pkt, 2 * pair + 1, :],
                                     pe[:, 512:1024],
                                     start=(pkt == first_kt),
                                     stop=last and not pend)

                for idx, kt in enumerate(active_kt):
                    k0 = kt * 128
                    scAB = ps_pool.tile([128, 1024], FP32, tag="ps")
                    nc.tensor.matmul(scAB[:, 0:512],
                                     kt_sb[0:64, pair, k0:k0 + 128],
                                     qt_sb[0:64, pair, q0:q0 + 512],
                                     start=True, stop=True)
                    nc.tensor.matmul(scAB[:, 512:1024],
                                     kt_sb[64:128, pair, k0:k0 + 128],
                                     qt_sb[64:128, pair, q0:q0 + 512],
                                     start=True, stop=True)
                    eAB = exp_pool.tile([128, 1024], FP16, tag="exp")
                    nc.scalar.activation(eAB[:], scAB[:], AFT.Exp,
                                         bias=kbias[:, kt:kt + 1], scale=0.125)
                    if len(pend) >= 2:
                        flush_pv()
                    pend.append((kt, eAB))
                    if idx == 1:
                        epi_burst()
                    if idx >= 1:
                        for _ in range(nsub):
                            next(g, None)
                while pend:
                    flush_pv(last=True)
                if is_ramp:
                    for _ in g:
                        pass
                qc_done(pair, q0, hA, hB)
            for _ in gen_rest:
                pass
        while pend_epi:
            epi_burst()

    nc.compile()
    return nc


_NC_CACHE = {}


def _get_nc(active_kt):
    key = tuple(active_kt)
    if key not in _NC_CACHE:
        _NC_CACHE[key] = build_kernel(key)
    return _NC_CACHE[key]


def _active_kt(mask):
    # key tile kt is skippable iff masked out in EVERY batch row (the
    # same compiled program runs on all cores).
    m = np.asarray(mask, dtype=np.float32).reshape(B, ST, 128)
    return tuple(t for t in range(ST) if np.any(m[:, t, :] != 0.0))


def _pack_w(W, cs):
    # [D, DC] -> [128, KT_D*DC]: partition p holds rows {kt*128+p}
    w = np.asarray(W)[:, cs].astype(np.float16)
    return np.ascontiguousarray(
        w.reshape(KT_D, 128, DC).transpose(1, 0, 2).reshape(128, KT_D * DC))


def make_in_maps(x, mask, Wq, bq, Wk, bk, Wv, bv):
    asc = np.ascontiguousarray
    x = np.asarray(x)
    mask = np.asarray(mask)
    in_maps = []
    for c in range(NCORES):
        b, g = divmod(c, 2)
        cs = slice(g * DC, (g + 1) * DC)
        xt = x[b].T.astype(np.float16)  # [D, S]
        xt = xt.reshape(KT_D, 128, QCH, 512).transpose(2, 1, 0, 3)
        in_maps.append({
            "xt": asc(xt.reshape(QCH, 128, KT_D * 512)),
            "mask": asc(mask[b].reshape(ST, 128).T, dtype=np.float32),
            "wq": _pack_w(Wq, cs),
            "wk": _pack_w(Wk, cs),
            "wv": _pack_w(Wv, cs),
            "bq": asc(np.asarray(bq)[cs].reshape(MT, 128).T, dtype=np.float32),
            "bk": asc(np.asarray(bk)[cs].reshape(MT, 128).T, dtype=np.float32),
            "bv": asc(bv[cs], dtype=np.float32),
        })
    return in_maps


def kernel(x, mask, Wq, bq, Wk, bk, Wv, bv):
    nc = _get_nc(_active_kt(mask))
    in_maps = make_in_maps(x, mask, Wq, bq, Wk, bk, Wv, bv)
    res = run_bass_kernel_spmd(nc, in_maps, core_ids=list(range(NCORES)))
    out = np.empty((B, S, D), dtype=np.float32)
    for c in range(NCORES):
        b, g = divmod(c, 2)
        out[b, :, g * DC:(g + 1) * DC] = res.results[c]["out"]
    return out


# revision 11
# speedup vs baseline: 1.1962x; 1.0359x over previous
"""Multi-head attention layer for Trainium2, 8 NeuronCores.

Problem (hardcoded): B=4, S=2048, D=1024, H=16 heads, DH=64.
  q,k,v = x@W* + b*;  scores = (q k^T)/sqrt(DH) - 10000*(1-mask_k);
  out = softmax(scores) @ v, heads concatenated.

Sharding: 8 cores = (batch b in 0..3) x (head-group g in 0..1).
Each core handles one batch element and 8 heads (512 of the 1024 output
channels), so outputs are disjoint and no collectives are needed.

Host-side prep (numpy layout work, not device FLOPs): x is transposed,
cast to fp16 and laid out chunk-major so every DMA is a contiguous
per-partition stream; W* likewise. This removes all on-device PE
transposes/casts and makes input DMA run at full HBM bandwidth.

Per-core kernel (the scalar engine's Exp at 1 elem/cycle/lane is the
bottleneck; all PE/DVE work is scheduled to hide under it):
  1. V' [s, dout|1] = xT.T @ Wv (+bias via rank-1 ones matmul; the ones
     column accumulates the softmax denominator during PV).
     QT/KT [dout, s] = W.T @ xT (bias added on PSUM->SBUF copy-out).
     Only the first half of V and K/Q for the first queries are
     projected up front; the rest streams inside early attention.
  2. Per head-pair, per 512-query chunk, per 128-key tile kt (fully
     masked key tiles are skipped - their exp is exactly 0):
     scoresT[k,q] for both heads in one PE pass (row-group tiling);
     expT = Exp(0.125*scoresT + maskbias_k) on the scalar engine;
     h'T[dd,q] += V'[k,dd].T @ expT  (row 64 = sum of exp = denom).
  3. h'T transposed back on the PE (spread one query-tile per kt step
     of the next chunk); h = h'T[0:64]/h'T[64], both heads coalesced
     into one [128,128] tile per query tile, DMA'd out.
"""
import numpy as np
from contextlib import ExitStack
from collections import deque
from itertools import chain

import concourse.bass as bass
import concourse.bacc as bacc
import concourse.mybir as mybir
from concourse.tile import TileContext
from concourse.bass_utils import run_bass_kernel_spmd
from concourse.masks import make_identity

B, S, D, H = 4, 2048, 1024, 16
DH = 64
HPC = 8            # heads per core
DC = HPC * DH      # 512 output channels per core
KT_D = D // 128    # 8 contraction tiles over d_in
MT = DC // 128     # 4 tiles over local d_out
ST = S // 128      # 16 s-tiles
QCH = S // 512     # 4 query chunks
NCORES = 8

FP32 = mybir.dt.float32
FP16 = mybir.dt.float16
AFT = mybir.ActivationFunctionType


def build_kernel(active_kt):
    """active_kt: sorted tuple of key-tile indices with any unmasked key.
    Fully-masked tiles contribute exp(s-10000)==0 exactly, so they are
    skipped in scores/exp/PV and V projection."""
    active_kt = list(active_kt)
    nc = bacc.Bacc("TRN2", target_bir_lowering=False, debug=False)
    # host-packed layouts (see make_in_maps)
    xt_d = nc.dram_tensor("xt", (QCH, 128, KT_D * 512), FP16, kind="ExternalInput")
    mask_d = nc.dram_tensor("mask", (128, ST), FP32, kind="ExternalInput")
    wq_d = nc.dram_tensor("wq", (128, KT_D * DC), FP16, kind="ExternalInput")
    wk_d = nc.dram_tensor("wk", (128, KT_D * DC), FP16, kind="ExternalInput")
    wv_d = nc.dram_tensor("wv", (128, KT_D * DC), FP16, kind="ExternalInput")
    bq_d = nc.dram_tensor("bq", (128, MT), FP32, kind="ExternalInput")
    bk_d = nc.dram_tensor("bk", (128, MT), FP32, kind="ExternalInput")
    bv_d = nc.dram_tensor("bv", (DC,), FP32, kind="ExternalInput")
    out_d = nc.dram_tensor("out", (S, DC), FP32, kind="ExternalOutput")

    with TileContext(nc) as tc, ExitStack() as ctx:
        const = ctx.enter_context(tc.tile_pool(name="const", bufs=1))
        big = ctx.enter_context(tc.tile_pool(name="big", bufs=1))
        exp_pool = ctx.enter_context(tc.tile_pool(name="expp", bufs=5))
        ht_pool = ctx.enter_context(tc.tile_pool(name="htp", bufs=2))
        o_pool = ctx.enter_context(tc.tile_pool(name="op", bufs=2))
        ps_pool = ctx.enter_context(
            tc.tile_pool(name="psp", bufs=2, space=bass.MemorySpace.PSUM))
        psh_pool = ctx.enter_context(
            tc.tile_pool(name="pshp", bufs=2, space=bass.MemorySpace.PSUM))
        pst_pool = ctx.enter_context(
            tc.tile_pool(name="pstp", bufs=2, space=bass.MemorySpace.PSUM))

        # ---- input DMAs (queue order: wv + x chunk 0 first so V
        # projection starts immediately) ----
        mask_sb = const.tile([128, ST], FP32)
        nc.sync.dma_start(mask_sb[:], mask_d[:])
        bq_sb = const.tile([128, MT], FP32)
        bk_sb = const.tile([128, MT], FP32)
        bv_f = const.tile([1, DC], FP32)
        nc.sync.dma_start(bq_sb[:], bq_d[:])
        nc.sync.dma_start(bk_sb[:], bk_d[:])
        nc.sync.dma_start(bv_f[:], bv_d[None, :])

        wv_sb = big.tile([128, KT_D, DC], FP16)
        wk_sb = big.tile([128, KT_D, DC], FP16)
        wq_sb = big.tile([128, KT_D, DC], FP16)
        xt_sb = big.tile([128, KT_D, S], FP16)
        half = KT_D // 2
        for h0 in (0, half):
            nc.sync.dma_start(
                wv_sb[:, h0:h0 + half, :],
                wv_d[:, h0 * DC:(h0 + half) * DC].rearrange(
                    "p (k n) -> p k n", n=DC))
            nc.sync.dma_start(
                xt_sb[:, h0:h0 + half, 0:512],
                xt_d[0, :, h0 * 512:(h0 + half) * 512].rearrange(
                    "p (k s) -> p k s", s=512))
        nc.sync.dma_start(wk_sb[:], wk_d[:].rearrange("p (k n) -> p k n", n=DC))
        nc.sync.dma_start(wq_sb[:], wq_d[:].rearrange("p (k n) -> p k n", n=DC))
        for qch in range(1, QCH):
            nc.sync.dma_start(
                xt_sb[:, :, qch * 512:(qch + 1) * 512],
                xt_d[qch].rearrange("p (k s) -> p k s", s=512))

        # ---- consts ----
        ident = const.tile([128, 128], FP32)
        make_identity(nc, ident[:])
        ones_r = const.tile([1, 128], FP16)
        nc.vector.memset(ones_r[:], 1.0)
        bv_row = const.tile([1, DC], FP16)
        nc.vector.tensor_copy(bv_row[:], bv_f[:])
        # additive bias per key position: -10000*(1-mask)
        kbias = const.tile([128, ST], FP32)
        nc.vector.tensor_scalar(kbias[:], mask_sb[:], -1.0, 10000.0,
                                mybir.AluOpType.add, mybir.AluOpType.mult)

        # persistent activations
        qt_sb = big.tile([128, MT, S], FP16)           # QT: [dout, s]
        kt_sb = big.tile([128, MT, S], FP16)           # KT: [dout, s]
        v_sb = big.tile([128, ST, HPC, DH + 1], FP16)  # V': [s_p, s_t, head, d|1]
        nc.vector.memset(v_sb[:, :, :, DH:DH + 1], 1.0)

        def v_tile(st, pool_tag):
            pool = ps_pool if pool_tag == "ps" else pst_pool
            ps = pool.tile([128, 512], FP32, tag=pool_tag,
                           bufs=1 if pool_tag == "pacc" else None)
            for ktc in range(KT_D):
                nc.tensor.matmul(
                    ps[:],
                    xt_sb[:, ktc, st * 128:(st + 1) * 128],
                    wv_sb[:, ktc, :],
                    start=(ktc == 0), stop=False)
                if pool_tag == "pacc" and ktc in (2, 5):
                    yield
            nc.tensor.matmul(ps[:], ones_r[:], bv_row[:], start=False, stop=True)
            nc.vector.tensor_copy(
                v_sb[:, st, :, 0:DH],
                ps[:].rearrange("p (h d) -> p h d", d=DH))
            if pool_tag == "pacc":
                yield

        K, Q = 0, 1

        def proj_tile(mt, which, qch, pool_tag):
            pool = ps_pool if pool_tag == "ps" else pst_pool
            w_sb, b_sb, dst = ((wk_sb, bk_sb, kt_sb), (wq_sb, bq_sb, qt_sb))[which]
            ps = pool.tile([128, 512], FP32, tag=pool_tag,
                           bufs=1 if pool_tag == "pacc" else None)
            for ktc in range(KT_D):
                nc.tensor.matmul(
                    ps[:],
                    w_sb[:, ktc, mt * 128:(mt + 1) * 128],
                    xt_sb[:, ktc, qch * 512:(qch + 1) * 512],
                    start=(ktc == 0), stop=(ktc == KT_D - 1))
                if pool_tag == "pacc" and ktc in (2, 5):
                    yield
            nc.vector.tensor_scalar_add(
                dst[:, mt, qch * 512:(qch + 1) * 512],
                ps[:], b_sb[:, mt:mt + 1])
            if pool_tag == "pacc":
                yield

        # ---- up-front projections: V first 6 tiles, K(0,0), Q(0,0) ----
        upfront_v = [st for st in active_kt if st < 6]
        stream_v = [st for st in active_kt if st >= 6]
        for st in upfront_v:
            for _ in v_tile(st, "ps"):
                pass
        for _ in proj_tile(0, K, 0, "ps"):
            pass
        for _ in proj_tile(0, Q, 0, "ps"):
            pass

        # streamed work (generators; "pacc" psum accumulator, bufs=1):
        #  qc0 of pair 0: rest of V + K(0,2..3) + Q(0,1..3), availability-
        #  ordered. pair p: K/Q of pair p+1.
        def ramp_gen():
            items = []
            sv = list(stream_v)
            items += [("v", st) for st in sv[0:2]]
            items.append(("p", (0, K, 1)))
            items += [("v", st) for st in sv[2:4]]
            items.append(("p", (0, K, 2)))
            items += [("v", st) for st in sv[4:7]]
            items.append(("p", (0, K, 3)))
            items += [("v", st) for st in sv[7:]]
            items += [("p", (0, Q, c)) for c in range(1, QCH)]
            for kind, arg in items:
                if kind == "v":
                    yield from v_tile(arg, "pacc")
                else:
                    yield from proj_tile(*arg, "pacc")

        def pair_gen(p):
            for which in (K, Q):
                for c in range(QCH):
                    yield from proj_tile(p, which, c, "pacc")

        # ---- deferred epilogue: at qc end, copy h' to SBUF; the
        # transpose/divide/DMA pieces run one query-tile per kt step of
        # the NEXT qc so the PE burst never starves the scalar engine.
        tpq = pst_pool.tile([128, 4, DH + 1], FP32, tag="tpq", bufs=1)
        epi_q = deque()

        def push_epilogue(epair, eq0, htA, htB):
            # 10 single-step closures: t,t,t,t,D,t,t,t,t,D
            def mk_tp(slot, ht, qt):
                def f():
                    nc.tensor.transpose(
                        tpq[:, slot, :], ht[:, qt * 128:(qt + 1) * 128],
                        ident[0:DH + 1, 0:DH + 1])
                return f

            def mk_dve(qt0):
                def f():
                    rec4 = o_pool.tile([128, 4], FP32, tag="rec")
                    nc.vector.reciprocal(rec4[:], tpq[:, :, DH])
                    for j, qt in ((0, qt0), (2, qt0 + 1)):
                        o_sb = o_pool.tile([128, 128], FP32, tag="o")
                        nc.vector.tensor_scalar_mul(
                            o_sb[:, 0:DH], tpq[:, j, 0:DH], rec4[:, j:j + 1])
                        nc.vector.tensor_scalar_mul(
                            o_sb[:, DH:128], tpq[:, j + 1, 0:DH],
                            rec4[:, j + 1:j + 2])
                        row = eq0 + qt * 128
                        nc.sync.dma_start(
                            out_d[row:row + 128,
                                  epair * 128:(epair + 1) * 128], o_sb[:])
                return f

            for qt0 in (0, 2):
                for slot in range(4):
                    epi_q.append(
                        mk_tp(slot, (htA, htB)[slot & 1], qt0 + (slot >> 1)))
                epi_q.append(mk_dve(qt0))

        def epi_burst():
            while epi_q:
                epi_q.popleft()()

        def qc_done(pair, q0, hA, hB):
            htA = ht_pool.tile([DH + 1, 512], FP32, tag="ht")
            nc.vector.tensor_copy(htA[:], hA[:])
            htB = ht_pool.tile([DH + 1, 512], FP32, tag="ht")
            nc.vector.tensor_copy(htB[:], hB[:])
            push_epilogue(pair, q0, htA, htB)

        first_kt = active_kt[0]
        for pair in range(HPC // 2):
            gen_rest = pair_gen(pair + 1) if pair < HPC // 2 - 1 else iter(())
            for qc in range(QCH):
                is_ramp = (pair == 0 and qc == 0)
                g = ramp_gen() if is_ramp else gen_rest
                nsub = 3 if is_ramp else 1
                q0 = qc * 512
                hA = psh_pool.tile([DH + 1, 512], FP32, tag="h")
                hB = psh_pool.tile([DH + 1, 512], FP32, tag="h")
                # software pipeline (depth 2): pv(kt-2) is emitted after
                # scores(kt) so the exp never waits behind PV/projection.
                pend = deque()

                def flush_pv(last=False):
                    pkt, pe = pend.popleft()
                    nc.tensor.matmul(hA[:], v_sb[:, pkt, 2 * pair, :],
                                     pe[:, 0:512],
                                     start=(pkt == first_kt),
                                     stop=last and not pend)
                    nc.tensor.matmul(hB[:], v_sb[:, pkt, 2 * pair + 1, :],
                                     pe[:, 512:1024],
                                     start=(pkt == first_kt),
                                     stop=last and not pend)

                for idx, kt in enumerate(active_kt):
                    k0 = kt * 128
                    scAB = ps_pool.tile([128, 1024], FP32, tag="ps")
                    nc.tensor.matmul(scAB[:, 0:512],
                                     kt_sb[0:64, pair, k0:k0 + 128],
                                     qt_sb[0:64, pair, q0:q0 + 512],
                                     start=True, stop=True)
                    nc.tensor.matmul(scAB[:, 512:1024],
                                     kt_sb[64:128, pair, k0:k0 + 128],
                                     qt_sb[64:128, pair, q0:q0 + 512],
                                     start=True, stop=True)
                    eAB = exp_pool.tile([128, 1024], FP16, tag="exp")
                    nc.scalar.activation(eAB[:], scAB[:], AFT.Exp,
                                         bias=kbias[:, kt:kt + 1], scale=0.125)
                    if len(pend) >= 2:
                        flush_pv()
                    pend.append((kt, eAB))
                    if idx >= 1 and epi_q:
                        epi_q.popleft()()
                    if idx >= 1:
                        for _ in range(nsub):
                            next(g, None)
                while pend:
                    flush_pv(last=True)
                epi_burst()
                if is_ramp:
                    for _ in g:
                        pass
                qc_done(pair, q0, hA, hB)
            for _ in gen_rest:
                pass
        epi_burst()

    nc.compile()
    return nc


_NC_CACHE = {}


def _get_nc(active_kt):
    key = tuple(active_kt)
    if key not in _NC_CACHE:
        _NC_CACHE[key] = build_kernel(key)
    return _NC_CACHE[key]


def _active_kt(mask):
    # key tile kt is skippable iff masked out in EVERY batch row (the
    # same compiled program runs on all cores).
    m = np.asarray(mask, dtype=np.float32).reshape(B, ST, 128)
    return tuple(t for t in range(ST) if np.any(m[:, t, :] != 0.0))


def _pack_w(W, cs):
    # [D, DC] -> [128, KT_D*DC]: partition p holds rows {kt*128+p}
    w = np.asarray(W)[:, cs].astype(np.float16)
    return np.ascontiguousarray(
        w.reshape(KT_D, 128, DC).transpose(1, 0, 2).reshape(128, KT_D * DC))


def make_in_maps(x, mask, Wq, bq, Wk, bk, Wv, bv):
    asc = np.ascontiguousarray
    x = np.asarray(x)
    mask = np.asarray(mask)
    in_maps = []
    for c in range(NCORES):
        b, g = divmod(c, 2)
        cs = slice(g * DC, (g + 1) * DC)
        xt = x[b].T.astype(np.float16)  # [D, S]
        xt = xt.reshape(KT_D, 128, QCH, 512).transpose(2, 1, 0, 3)
        in_maps.append({
            "xt": asc(xt.reshape(QCH, 128, KT_D * 512)),
            "mask": asc(mask[b].reshape(ST, 128).T, dtype=np.float32),
            "wq": _pack_w(Wq, cs),
            "wk": _pack_w(Wk, cs),
            "wv": _pack_w(Wv, cs),
            "bq": asc(np.asarray(bq)[cs].reshape(MT, 128).T, dtype=np.float32),
            "bk": asc(np.asarray(bk)[cs].reshape(MT, 128).T, dtype=np.float32),
            "bv": asc(bv[cs], dtype=np.float32),
        })
    return in_maps


def kernel(x, mask, Wq, bq, Wk, bk, Wv, bv):
    nc = _get_nc(_active_kt(mask))
    in_maps = make_in_maps(x, mask, Wq, bq, Wk, bk, Wv, bv)
    res = run_bass_kernel_spmd(nc, in_maps, core_ids=list(range(NCORES)))
    out = np.empty((B, S, D), dtype=np.float32)
    for c in range(NCORES):
        b, g = divmod(c, 2)
        out[b, :, g * DC:(g + 1) * DC] = res.results[c]["out"]
    return out


# revision 12
# speedup vs baseline: 1.2124x; 1.0136x over previous
"""Multi-head attention layer for Trainium2, 8 NeuronCores.

Problem (hardcoded): B=4, S=2048, D=1024, H=16 heads, DH=64.
  q,k,v = x@W* + b*;  scores = (q k^T)/sqrt(DH) - 10000*(1-mask_k);
  out = softmax(scores) @ v, heads concatenated.

Sharding: 8 cores = (batch b in 0..3) x (head-group g in 0..1).
Each core handles one batch element and 8 heads (512 of the 1024 output
channels), so outputs are disjoint and no collectives are needed.

Host-side prep (numpy layout work, not device FLOPs): x is transposed,
cast to fp16 and laid out chunk-major so every DMA is a contiguous
per-partition stream; W* likewise. This removes all on-device PE
transposes/casts and makes input DMA run at full HBM bandwidth.

Per-core kernel (the scalar engine's Exp at 1 elem/cycle/lane is the
bottleneck; all PE/DVE work is scheduled to hide under it):
  1. V' [s, dout|1] = xT.T @ Wv (+bias via rank-1 ones matmul; the ones
     column accumulates the softmax denominator during PV).
     QT/KT [dout, s] = W.T @ xT (bias added on PSUM->SBUF copy-out).
     Only the first half of V and K/Q for the first queries are
     projected up front; the rest streams inside early attention.
  2. Per head-pair, per 512-query chunk, per 128-key tile kt (fully
     masked key tiles are skipped - their exp is exactly 0):
     scoresT[k,q] for both heads in one PE pass (row-group tiling);
     expT = Exp(0.125*scoresT + maskbias_k) on the scalar engine;
     h'T[dd,q] += V'[k,dd].T @ expT  (row 64 = sum of exp = denom).
  3. h'T transposed back on the PE (spread one query-tile per kt step
     of the next chunk); h = h'T[0:64]/h'T[64], both heads coalesced
     into one [128,128] tile per query tile, DMA'd out.
"""
import numpy as np
from contextlib import ExitStack
from collections import deque
from itertools import chain

import concourse.bass as bass
import concourse.bacc as bacc
import concourse.mybir as mybir
from concourse.tile import TileContext
from concourse.bass_utils import run_bass_kernel_spmd
from concourse.masks import make_identity

B, S, D, H = 4, 2048, 1024, 16
DH = 64
HPC = 8            # heads per core
DC = HPC * DH      # 512 output channels per core
KT_D = D // 128    # 8 contraction tiles over d_in
MT = DC // 128     # 4 tiles over local d_out
ST = S // 128      # 16 s-tiles
QCH = S // 512     # 4 query chunks
NCORES = 8

FP32 = mybir.dt.float32
FP16 = mybir.dt.float16
AFT = mybir.ActivationFunctionType


def build_kernel(active_kt):
    """active_kt: sorted tuple of key-tile indices with any unmasked key.
    Fully-masked tiles contribute exp(s-10000)==0 exactly, so they are
    skipped in scores/exp/PV and V projection."""
    active_kt = list(active_kt)
    nc = bacc.Bacc("TRN2", target_bir_lowering=False, debug=False)
    # host-packed layouts (see make_in_maps)
    xt_d = nc.dram_tensor("xt", (QCH, 128, KT_D * 512), FP16, kind="ExternalInput")
    mask_d = nc.dram_tensor("mask", (128, ST), FP32, kind="ExternalInput")
    wq_d = nc.dram_tensor("wq", (128, KT_D * DC), FP16, kind="ExternalInput")
    wk_d = nc.dram_tensor("wk", (128, KT_D * DC), FP16, kind="ExternalInput")
    wv_d = nc.dram_tensor("wv", (128, KT_D * DC), FP16, kind="ExternalInput")
    bq_d = nc.dram_tensor("bq", (128, MT), FP32, kind="ExternalInput")
    bk_d = nc.dram_tensor("bk", (128, MT), FP32, kind="ExternalInput")
    bv_d = nc.dram_tensor("bv", (DC,), FP32, kind="ExternalInput")
    out_d = nc.dram_tensor("out", (S, DC), FP32, kind="ExternalOutput")

    with TileContext(nc) as tc, ExitStack() as ctx:
        const = ctx.enter_context(tc.tile_pool(name="const", bufs=1))
        big = ctx.enter_context(tc.tile_pool(name="big", bufs=1))
        exp_pool = ctx.enter_context(tc.tile_pool(name="expp", bufs=5))
        ht_pool = ctx.enter_context(tc.tile_pool(name="htp", bufs=2))
        o_pool = ctx.enter_context(tc.tile_pool(name="op", bufs=2))
        ps_pool = ctx.enter_context(
            tc.tile_pool(name="psp", bufs=2, space=bass.MemorySpace.PSUM))
        psh_pool = ctx.enter_context(
            tc.tile_pool(name="pshp", bufs=2, space=bass.MemorySpace.PSUM))
        pst_pool = ctx.enter_context(
            tc.tile_pool(name="pstp", bufs=2, space=bass.MemorySpace.PSUM))

        # ---- input DMAs (queue order: wv + x chunk 0 first so V
        # projection starts immediately) ----
        mask_sb = const.tile([128, ST], FP32)
        nc.sync.dma_start(mask_sb[:], mask_d[:])
        bq_sb = const.tile([128, MT], FP32)
        bk_sb = const.tile([128, MT], FP32)
        bv_f = const.tile([1, DC], FP32)
        nc.sync.dma_start(bq_sb[:], bq_d[:])
        nc.sync.dma_start(bk_sb[:], bk_d[:])
        nc.sync.dma_start(bv_f[:], bv_d[None, :])

        wv_sb = big.tile([128, KT_D, DC], FP16)
        wk_sb = big.tile([128, KT_D, DC], FP16)
        wq_sb = big.tile([128, KT_D, DC], FP16)
        xt_sb = big.tile([128, KT_D, S], FP16)
        half = KT_D // 2
        for h0 in (0, half):
            nc.sync.dma_start(
                wv_sb[:, h0:h0 + half, :],
                wv_d[:, h0 * DC:(h0 + half) * DC].rearrange(
                    "p (k n) -> p k n", n=DC))
            nc.sync.dma_start(
                xt_sb[:, h0:h0 + half, 0:512],
                xt_d[0, :, h0 * 512:(h0 + half) * 512].rearrange(
                    "p (k s) -> p k s", s=512))
        nc.sync.dma_start(wk_sb[:], wk_d[:].rearrange("p (k n) -> p k n", n=DC))
        nc.sync.dma_start(wq_sb[:], wq_d[:].rearrange("p (k n) -> p k n", n=DC))
        for qch in range(1, QCH):
            nc.sync.dma_start(
                xt_sb[:, :, qch * 512:(qch + 1) * 512],
                xt_d[qch].rearrange("p (k s) -> p k s", s=512))

        # ---- consts ----
        ident = const.tile([128, 128], FP32)
        make_identity(nc, ident[:])
        ones_r = const.tile([1, 128], FP16)
        nc.vector.memset(ones_r[:], 1.0)
        bv_row = const.tile([1, DC], FP16)
        nc.vector.tensor_copy(bv_row[:], bv_f[:])
        # additive bias per key position: -10000*(1-mask)
        kbias = const.tile([128, ST], FP32)
        nc.vector.tensor_scalar(kbias[:], mask_sb[:], -1.0, 10000.0,
                                mybir.AluOpType.add, mybir.AluOpType.mult)

        # persistent activations
        qt_sb = big.tile([128, MT, S], FP16)           # QT: [dout, s]
        kt_sb = big.tile([128, MT, S], FP16)           # KT: [dout, s]
        v_sb = big.tile([128, ST, HPC, DH + 1], FP16)  # V': [s_p, s_t, head, d|1]
        nc.vector.memset(v_sb[:, :, :, DH:DH + 1], 1.0)

        def v_tile(st, pool_tag):
            pool = ps_pool if pool_tag == "ps" else pst_pool
            ps = pool.tile([128, 512], FP32, tag=pool_tag,
                           bufs=1 if pool_tag == "pacc" else None)
            for ktc in range(KT_D):
                nc.tensor.matmul(
                    ps[:],
                    xt_sb[:, ktc, st * 128:(st + 1) * 128],
                    wv_sb[:, ktc, :],
                    start=(ktc == 0), stop=False)
                if pool_tag == "pacc" and ktc in (2, 5):
                    yield
            nc.tensor.matmul(ps[:], ones_r[:], bv_row[:], start=False, stop=True)
            nc.vector.tensor_copy(
                v_sb[:, st, :, 0:DH],
                ps[:].rearrange("p (h d) -> p h d", d=DH))
            if pool_tag == "pacc":
                yield

        K, Q = 0, 1

        def proj_tile(mt, which, qch, pool_tag):
            pool = ps_pool if pool_tag == "ps" else pst_pool
            w_sb, b_sb, dst = ((wk_sb, bk_sb, kt_sb), (wq_sb, bq_sb, qt_sb))[which]
            ps = pool.tile([128, 512], FP32, tag=pool_tag,
                           bufs=1 if pool_tag == "pacc" else None)
            for ktc in range(KT_D):
                nc.tensor.matmul(
                    ps[:],
                    w_sb[:, ktc, mt * 128:(mt + 1) * 128],
                    xt_sb[:, ktc, qch * 512:(qch + 1) * 512],
                    start=(ktc == 0), stop=(ktc == KT_D - 1))
                if pool_tag == "pacc" and ktc in (2, 5):
                    yield
            nc.vector.tensor_scalar_add(
                dst[:, mt, qch * 512:(qch + 1) * 512],
                ps[:], b_sb[:, mt:mt + 1])
            if pool_tag == "pacc":
                yield

        # ---- up-front projections: V first 6 tiles, K(0,0), Q(0,0) ----
        upfront_v = [st for st in active_kt if st < 6]
        stream_v = [st for st in active_kt if st >= 6]
        for st in upfront_v:
            for _ in v_tile(st, "ps"):
                pass
        for _ in proj_tile(0, K, 0, "ps"):
            pass
        for _ in proj_tile(0, Q, 0, "ps"):
            pass

        # streamed work (generators; "pacc" psum accumulator, bufs=1):
        #  qc0 of pair 0: rest of V + K(0,2..3) + Q(0,1..3), availability-
        #  ordered. pair p: K/Q of pair p+1.
        def ramp_gen():
            items = []
            sv = list(stream_v)
            items += [("v", st) for st in sv[0:2]]
            items.append(("p", (0, K, 1)))
            items += [("v", st) for st in sv[2:4]]
            items.append(("p", (0, K, 2)))
            items += [("v", st) for st in sv[4:7]]
            items.append(("p", (0, K, 3)))
            items += [("v", st) for st in sv[7:]]
            items.append(("p", (0, Q, 1)))
            for kind, arg in items:
                if kind == "v":
                    yield from v_tile(arg, "pacc")
                else:
                    yield from proj_tile(*arg, "pacc")

        def pair_gen(p, pre=()):
            for (mt, which, c) in pre:
                yield from proj_tile(mt, which, c, "pacc")
            for which in (K, Q):
                for c in range(QCH):
                    yield from proj_tile(p, which, c, "pacc")

        # ---- deferred epilogue: at qc end, copy h' to SBUF; the
        # transpose/divide/DMA pieces run one query-tile per kt step of
        # the NEXT qc so the PE burst never starves the scalar engine.
        tpq = pst_pool.tile([128, 4, DH + 1], FP32, tag="tpq", bufs=1)
        epi_q = deque()

        def push_epilogue(epair, eq0, htA, htB):
            # 10 single-step closures: t,t,t,t,D,t,t,t,t,D
            def mk_tp(slot, ht, qt):
                def f():
                    nc.tensor.transpose(
                        tpq[:, slot, :], ht[:, qt * 128:(qt + 1) * 128],
                        ident[0:DH + 1, 0:DH + 1])
                return f

            def mk_dve(qt0):
                def f():
                    rec4 = o_pool.tile([128, 4], FP32, tag="rec")
                    nc.vector.reciprocal(rec4[:], tpq[:, :, DH])
                    for j, qt in ((0, qt0), (1, qt0 + 1)):
                        o_sb = o_pool.tile([128, 128], FP32, tag="o")
                        nc.vector.tensor_scalar_mul(
                            o_sb[:, 0:DH], tpq[:, j, 0:DH], rec4[:, j:j + 1])
                        nc.vector.tensor_scalar_mul(
                            o_sb[:, DH:128], tpq[:, j + 2, 0:DH],
                            rec4[:, j + 2:j + 3])
                        row = eq0 + qt * 128
                        nc.sync.dma_start(
                            out_d[row:row + 128,
                                  epair * 128:(epair + 1) * 128], o_sb[:])
                return f

            for qt0 in (0, 2):
                # slots: 0,1 = head A qt0/qt0+1; 2,3 = head B (A first so
                # B's transposes don't wait on the second SBUF copy)
                for slot in range(4):
                    epi_q.append(
                        mk_tp(slot, (htA, htB)[slot >> 1], qt0 + (slot & 1)))
                epi_q.append(mk_dve(qt0))

        def epi_burst():
            while epi_q:
                epi_q.popleft()()

        def qc_done(pair, q0, hA, hB):
            htA = ht_pool.tile([DH + 1, 512], FP32, tag="ht")
            nc.vector.tensor_copy(htA[:], hA[:])
            htB = ht_pool.tile([DH + 1, 512], FP32, tag="ht")
            nc.vector.tensor_copy(htB[:], hB[:])
            push_epilogue(pair, q0, htA, htB)

        first_kt = active_kt[0]
        for pair in range(HPC // 2):
            pre = (((0, Q, 2), (0, Q, 3)) if pair == 0 else ())
            gen_rest = (pair_gen(pair + 1, pre)
                        if pair < HPC // 2 - 1 else iter(()))
            for qc in range(QCH):
                is_ramp = (pair == 0 and qc == 0)
                g = ramp_gen() if is_ramp else gen_rest
                nsub = 3 if is_ramp else 1
                q0 = qc * 512
                hA = psh_pool.tile([DH + 1, 512], FP32, tag="h")
                hB = psh_pool.tile([DH + 1, 512], FP32, tag="h")
                # software pipeline (depth 2): pv(kt-2) is emitted after
                # scores(kt) so the exp never waits behind PV/projection.
                pend = deque()

                def flush_pv(last=False):
                    pkt, pe = pend.popleft()
                    nc.tensor.matmul(hA[:], v_sb[:, pkt, 2 * pair, :],
                                     pe[:, 0:512],
                                     start=(pkt == first_kt),
                                     stop=last and not pend)
                    nc.tensor.matmul(hB[:], v_sb[:, pkt, 2 * pair + 1, :],
                                     pe[:, 512:1024],
                                     start=(pkt == first_kt),
                                     stop=last and not pend)

                for idx, kt in enumerate(active_kt):
                    k0 = kt * 128
                    scAB = ps_pool.tile([128, 1024], FP32, tag="ps")
                    nc.tensor.matmul(scAB[:, 0:512],
                                     kt_sb[0:64, pair, k0:k0 + 128],
                                     qt_sb[0:64, pair, q0:q0 + 512],
                                     start=True, stop=True)
                    nc.tensor.matmul(scAB[:, 512:1024],
                                     kt_sb[64:128, pair, k0:k0 + 128],
                                     qt_sb[64:128, pair, q0:q0 + 512],
                                     start=True, stop=True)
                    eAB = exp_pool.tile([128, 1024], FP16, tag="exp")
                    nc.scalar.activation(eAB[:], scAB[:], AFT.Exp,
                                         bias=kbias[:, kt:kt + 1], scale=0.125)
                    if len(pend) >= 2:
                        flush_pv()
                    pend.append((kt, eAB))
                    if idx >= 5 and epi_q:
                        epi_q.popleft()()
                    if idx >= 1:
                        for _ in range(nsub):
                            next(g, None)
                while pend:
                    flush_pv(last=True)
                epi_burst()
                if is_ramp:
                    for _ in g:
                        pass
                qc_done(pair, q0, hA, hB)
            for _ in gen_rest:
                pass
        epi_burst()

    nc.compile()
    return nc


_NC_CACHE = {}


def _get_nc(active_kt):
    key = tuple(active_kt)
    if key not in _NC_CACHE:
        _NC_CACHE[key] = build_kernel(key)
    return _NC_CACHE[key]


def _active_kt(mask):
    # key tile kt is skippable iff masked out in EVERY batch row (the
    # same compiled program runs on all cores).
    m = np.asarray(mask, dtype=np.float32).reshape(B, ST, 128)
    return tuple(t for t in range(ST) if np.any(m[:, t, :] != 0.0))


def _pack_w(W, cs):
    # [D, DC] -> [128, KT_D*DC]: partition p holds rows {kt*128+p}
    w = np.asarray(W)[:, cs].astype(np.float16)
    return np.ascontiguousarray(
        w.reshape(KT_D, 128, DC).transpose(1, 0, 2).reshape(128, KT_D * DC))


def make_in_maps(x, mask, Wq, bq, Wk, bk, Wv, bv):
    asc = np.ascontiguousarray
    x = np.asarray(x)
    mask = np.asarray(mask)
    in_maps = []
    for c in range(NCORES):
        b, g = divmod(c, 2)
        cs = slice(g * DC, (g + 1) * DC)
        xt = x[b].T.astype(np.float16)  # [D, S]
        xt = xt.reshape(KT_D, 128, QCH, 512).transpose(2, 1, 0, 3)
        in_maps.append({
            "xt": asc(xt.reshape(QCH, 128, KT_D * 512)),
            "mask": asc(mask[b].reshape(ST, 128).T, dtype=np.float32),
            "wq": _pack_w(Wq, cs),
            "wk": _pack_w(Wk, cs),
            "wv": _pack_w(Wv, cs),
            "bq": asc(np.asarray(bq)[cs].reshape(MT, 128).T, dtype=np.float32),
            "bk": asc(np.asarray(bk)[cs].reshape(MT, 128).T, dtype=np.float32),
            "bv": asc(bv[cs], dtype=np.float32),
        })
    return in_maps


def kernel(x, mask, Wq, bq, Wk, bk, Wv, bv):
    nc = _get_nc(_active_kt(mask))
    in_maps = make_in_maps(x, mask, Wq, bq, Wk, bk, Wv, bv)
    res = run_bass_kernel_spmd(nc, in_maps, core_ids=list(range(NCORES)))
    out = np.empty((B, S, D), dtype=np.float32)
    for c in range(NCORES):
        b, g = divmod(c, 2)
        out[b, :, g * DC:(g + 1) * DC] = res.results[c]["out"]
    return out
